# revision 2
# baseline (speedup 1.0000x reference)
"""Decoder attention (QKV proj + KV-cache scatter + softmax attention + out
proj) on 8 trn2 cores — v3.

Core = (batch b, head-group g): b = core//2, g = core%2; 8 heads per core.
Softmax over a permutation of the key axis: attend over concat([k_new,
cache_keep]) — no on-device scatter (complement rows gathered host-side).

All matmuls bf16 (fp32 PSUM). Engine schedule built from HW microbenchmarks:
  - Scores (contraction DH=64) run as ROW-TILED CONCURRENT head pairs: heads
    2hp/2hp+1 live on partitions 0-63/64-127 of the q/k tiles, so their MMs
    land on PE row-groups 0-1/2-3 and overlap (measured 114 ns/MM vs 448
    serial, N=512).
  - All chained matmuls order chunk-columns INNERMOST so each stationary
    serves 2 adjacent MMs — amortizes LDWEIGHTS (measured 229 vs 302 ns/MM).
  - exp on ACT: 2 tiles of [128,1024] per (pair, jt) back to back.
  - attn@V with a ones-column appended to V (VW=65): softmax denominator
    accumulates free in the same PSUM tile; per-query reciprocal broadcast
    via gpsimd; DVE multiplies straight out of PSUM (no staging copy).

PSUM: shared [128,1024] f32 pool bufs=2 (4 banks) for scores/qkv/vproj/proj
+ avA/avB accumulators [65,1024] (2+2 banks) = 8 banks exactly.
"""

import sys
import os

for _p in ("/opt/trn_rl_repo", "/root/.axon_site/_ro/trn_rl_repo"):
    if os.path.isdir(_p) and _p not in sys.path:
        sys.path.insert(0, _p)
        break

import numpy as np
import ml_dtypes

import concourse.bacc as bacc
import concourse.mybir as mybir
import concourse.tile as tile
from concourse import bass_utils

B, NX, NC, C, H = 4, 1024, 2048, 1024, 16
DH = C // H                      # 64
G = 2                            # head groups
HPG = H // G                     # 8 heads per group
CG = HPG * DH                    # 512 channels per group
SCALE = DH ** -0.5
N_CORES = 8
NJ = NC // 128                   # 16 j-tiles
VW = DH + 1                      # 65
F32 = mybir.dt.float32
BF16 = mybir.dt.bfloat16
EXP = mybir.ActivationFunctionType.Exp

_STATE = {}


def _build(reps: int = 1):
    nc = bacc.Bacc("TRN2", target_bir_lowering=False, debug=False)

    xT_d = nc.dram_tensor("xT", [C, NX], BF16, kind="ExternalInput")
    wqkT_d = nc.dram_tensor("wqkT", [C, 2 * CG], BF16, kind="ExternalInput")
    wvT_d = nc.dram_tensor("wvT", [C, CG], BF16, kind="ExternalInput")
    bqk_d = nc.dram_tensor("bqk", [128, 8], F32, kind="ExternalInput")
    bv_d = nc.dram_tensor("bv", [128, CG], F32, kind="ExternalInput")
    kkeepT_d = nc.dram_tensor("kkeepT", [CG, NC - NX], BF16, kind="ExternalInput")
    vkeep_d = nc.dram_tensor("vkeep", [NC - NX, HPG * VW], BF16, kind="ExternalInput")
    wprojT_d = nc.dram_tensor("wprojT", [CG, C], BF16, kind="ExternalInput")
    ones_d = nc.dram_tensor("ones8", [128, 8], BF16, kind="ExternalInput")
    outT_d = nc.dram_tensor("outT", [C, NX], BF16, kind="ExternalOutput")

    with tile.TileContext(nc) as tc:
        with (
            tc.tile_pool(name="persist", bufs=1) as pp,
            tc.tile_pool(name="work", bufs=1) as wp,
            tc.tile_pool(name="wqkc", bufs=16) as wqkp,
            tc.tile_pool(name="etA", bufs=3) as epA,
            tc.tile_pool(name="etB", bufs=3) as epB,
            tc.tile_pool(name="nrm", bufs=2) as np_pool,
            tc.tile_pool(name="out_sb", bufs=1) as op,
            tc.tile_pool(name="ps", bufs=1, space="PSUM") as psp,
        ):
            q_t = [pp.tile([128, NX], BF16, tag=f"q{i}", name=f"q{i}") for i in range(4)]
            k_t = [pp.tile([128, NC], BF16, tag=f"k{i}", name=f"k{i}") for i in range(4)]
            v_t = [pp.tile([128, HPG * VW], BF16, tag=f"v{i}", name=f"v{i}")
                   for i in range(NJ)]
            a_t = [pp.tile([128, NX], BF16, tag=f"a{i}", name=f"a{i}") for i in range(4)]
            wpj_t = [pp.tile([128, NX], BF16, tag=f"wp{i}", name=f"wp{i}")
                     for i in range(4)]
            bqk_t = pp.tile([128, 8], F32, tag="bqk")
            bv_t = pp.tile([128, CG], F32, tag="bv")
            xT_t = [wp.tile([128, NX], BF16, tag=f"x{i}", name=f"x{i}") for i in range(8)]
            wv_t = [wp.tile([128, CG], BF16, tag=f"wv{i}", name=f"wv{i}")
                    for i in range(8)]

            nc.sync.dma_start(bqk_t[:], bqk_d.ap())
            nc.sync.dma_start(bv_t[:], bv_d.ap())

            def qk_mtile(m, dma_only=False, chunks=None):
                """m 0-3: q m-tile; 4-7: k m-tile. Chunk-cols innermost so
                each wqk chunk stationary serves 2 adjacent MMs."""
                if chunks is None:
                    chunks = []
                    for kk in range(8):
                        wc = wqkp.tile([128, 128], BF16, tag="wqkc",
                                       name=f"wqkc{m}_{kk}")
                        nc.sync.dma_start(
                            wc[:],
                            wqkT_d[kk * 128:(kk + 1) * 128, m * 128:(m + 1) * 128])
                        chunks.append(wc)
                    if dma_only:
                        return chunks
                qps = psp.tile([128, NX], F32, tag="big", bufs=2, name=f"qps{m}")
                for kk in range(8):
                    for cch in range(2):
                        nc.tensor.matmul(
                            qps[:, cch * 512:(cch + 1) * 512],
                            chunks[kk][:],
                            xT_t[kk][:, cch * 512:(cch + 1) * 512],
                            start=(kk == 0), stop=(kk == 7),
                        )
                dest = q_t[m][:] if m < 4 else k_t[m - 4][:, 0:NX]
                nc.vector.tensor_scalar_add(dest, qps[:], bqk_t[:, m:m + 1])
                return None

            def body():
                # ---- DMAs ordered by first use; first q m-tile jumps queue ----
                nc.sync.dma_start(xT_t[0][:], xT_d[0:128, :])
                m0_chunks = qk_mtile(0, dma_only=True)
                for i in range(1, 8):
                    nc.sync.dma_start(xT_t[i][:], xT_d[i * 128:(i + 1) * 128, :])
                nc.sync.dma_start(k_t[0][:, NX:NC], kkeepT_d[0:128, :])
                qk_mtile(0, chunks=m0_chunks)

                # bulk loads, behind the critical path
                for m in range(NJ // 2):
                    nc.sync.dma_start(
                        v_t[m][:].rearrange("p (h w) -> p h w", w=VW)[:, :, DH],
                        ones_d.ap(),
                    )
                for i in range(8):
                    nc.sync.dma_start(wv_t[i][:], wvT_d[i * 128:(i + 1) * 128, :])
                for j in range(NJ // 2, NJ):
                    r0 = (j - NJ // 2) * 128
                    nc.sync.dma_start(v_t[j][:], vkeep_d[r0:r0 + 128, :])
                for i in range(1, 4):
                    nc.sync.dma_start(k_t[i][:, NX:NC],
                                      kkeepT_d[i * 128:(i + 1) * 128, :])
                for kk in range(4):
                    nc.sync.dma_start(wpj_t[kk][:],
                                      wprojT_d[kk * 128:(kk + 1) * 128, :])

                # ---- background thunks ----
                def qk_thunks(m):
                    chunks = []

                    def dmas(m=m):
                        chunks.extend(qk_mtile(m, dma_only=True))
                    yield dmas

                    def mms(m=m):
                        qk_mtile(m, chunks=chunks)
                    yield mms

                def v_thunks(m):
                    def mms(m=m):
                        vps = psp.tile([128, NX], F32, tag="big", bufs=2,
                                       name=f"vps{m}")
                        for kk in range(8):
                            nc.tensor.matmul(
                                vps[:, 0:CG],
                                xT_t[kk][:, m * 128:(m + 1) * 128],
                                wv_t[kk][:],
                                start=(kk == 0), stop=(kk == 7),
                            )
                        for h in range(HPG):
                            nc.vector.tensor_add(
                                v_t[m][:, h * VW:h * VW + DH],
                                vps[:, h * DH:(h + 1) * DH],
                                bv_t[:, h * DH:(h + 1) * DH],
                            )
                    yield mms

                pending = []
                pending.extend(qk_thunks(4))
                for m in range(8):
                    pending.extend(v_thunks(m))
                for m in (1, 5, 2, 6, 3, 7):
                    pending.extend(qk_thunks(m))

                def drain(n):
                    for _ in range(n):
                        if pending:
                            pending.pop(0)()

                # ---- attention on head pairs (rowtile concurrency) ----
                def attn_pair(hp, j_order, drain_per_j=1):
                    avA = psp.tile([VW, NX], F32, tag="avA", bufs=1, name=f"avA{hp}")
                    avB = psp.tile([VW, NX], F32, tag="avB", bufs=1, name=f"avB{hp}")
                    ets = {}

                    def emit_av(j, first, last):
                        for idx, av in ((0, avA), (1, avB)):
                            et = ets[(idx, j)]
                            for cch in range(2):
                                nc.tensor.matmul(
                                    av[:, cch * 512:(cch + 1) * 512],
                                    v_t[j][:, (2 * hp + idx) * VW:(2 * hp + idx + 1) * VW],
                                    et[:, cch * 512:(cch + 1) * 512],
                                    start=first, stop=last,
                                )

                    for step, j in enumerate(j_order):
                        sA = psp.tile([128, NX], F32, tag="big", bufs=2,
                                      name=f"sA{hp}_{j}")
                        sB = psp.tile([128, NX], F32, tag="big", bufs=2,
                                      name=f"sB{hp}_{j}")
                        # interleave A/B chunks: adjacent MMs sit on different
                        # PE row groups and run concurrently
                        for cch in range(2):
                            nc.tensor.matmul(
                                sA[:, cch * 512:(cch + 1) * 512],
                                k_t[hp][0:64, j * 128:(j + 1) * 128],
                                q_t[hp][0:64, cch * 512:(cch + 1) * 512],
                                start=True, stop=True,
                            )
                            nc.tensor.matmul(
                                sB[:, cch * 512:(cch + 1) * 512],
                                k_t[hp][64:128, j * 128:(j + 1) * 128],
                                q_t[hp][64:128, cch * 512:(cch + 1) * 512],
                                start=True, stop=True,
                            )
                        etA = epA.tile([128, NX], BF16, tag="etA", name=f"etA{hp}_{j}")
                        etB = epB.tile([128, NX], BF16, tag="etB", name=f"etB{hp}_{j}")
                        ets[(0, j)] = etA
                        ets[(1, j)] = etB
                        nc.scalar.activation(etA[:], sA[:], EXP, scale=SCALE)
                        nc.scalar.activation(etB[:], sB[:], EXP, scale=SCALE)
                        drain(drain_per_j)
                        if step >= 1:
                            emit_av(j_order[step - 1], step == 1, False)
                    emit_av(j_order[-1], False, True)

                    for idx, av in ((0, avA), (1, avB)):
                        recip = np_pool.tile([1, NX], F32, tag="recip", bufs=2)
                        nc.vector.reciprocal(recip[:], av[DH:VW, :])
                        rb = np_pool.tile([64, NX], F32, tag="rb", bufs=2)
                        nc.gpsimd.partition_broadcast(rb[:], recip[:])
                        nc.vector.tensor_mul(
                            a_t[hp][idx * 64:(idx + 1) * 64, :],
                            av[0:DH, :], rb[:])

                # keep-half pairs first: their V comes from DMA while the
                # new-half V projection drains in the background
                for hp in range(4):
                    attn_pair(hp, list(range(NJ // 2, NJ)) + list(range(NJ // 2)),
                              drain_per_j=1)
                drain(100)

                # ---- output projection (partial; host sums the two groups) ----
                for m in range(8):
                    pps = psp.tile([128, NX], F32, tag="big", bufs=2, name=f"pps{m}")
                    for kk in range(4):
                        for cch in range(2):
                            nc.tensor.matmul(
                                pps[:, cch * 512:(cch + 1) * 512],
                                wpj_t[kk][:, m * 128:(m + 1) * 128],
                                a_t[kk][:, cch * 512:(cch + 1) * 512],
                                start=(kk == 0), stop=(kk == 3),
                            )
                    ot = op.tile([128, NX], BF16, tag="ot", bufs=2)
                    nc.vector.tensor_copy(ot[:], pps[:])
                    nc.sync.dma_start(outT_d[m * 128:(m + 1) * 128, :], ot[:])

            if reps == 1:
                body()
            else:
                hints = (
                    mybir.EngineType.PE,
                    mybir.EngineType.Activation,
                    mybir.EngineType.DVE,
                    mybir.EngineType.SP,
                    mybir.EngineType.Pool,
                )
                with tc.For_i(0, reps, 1, hint_engines=hints):
                    body()

    nc.compile()
    return nc


def _prep_in_maps(x, update_idx, cache_k, cache_v, w_qkv, b_qkv):
    """Host-side sharding: 8 per-core input dicts (bf16 activations/weights)."""
    BF = ml_dtypes.bfloat16
    x = np.asarray(x, np.float32)
    update_idx = np.asarray(update_idx)
    cache_k = np.asarray(cache_k, np.float32)
    cache_v = np.asarray(cache_v, np.float32)
    w_qkv = np.asarray(w_qkv, np.float32)
    b_qkv = np.asarray(b_qkv, np.float32)

    per_g = []
    for g in range(G):
        qs = slice(g * CG, (g + 1) * CG)
        ks = slice(C + g * CG, C + (g + 1) * CG)
        vs = slice(2 * C + g * CG, 2 * C + (g + 1) * CG)
        wqkT = np.ascontiguousarray(
            np.concatenate([w_qkv[qs], w_qkv[ks]], 0).T).astype(BF)
        wvT = np.ascontiguousarray(w_qkv[vs].T).astype(BF)
        bqk = np.ascontiguousarray(
            np.concatenate([b_qkv[qs], b_qkv[ks]]).reshape(8, 128).T)
        bv = np.broadcast_to(b_qkv[vs][None, :], (128, CG)).copy()
        wprojT = np.ascontiguousarray(
            np.asarray(_STATE["w_proj"], np.float32)[:, g * CG:(g + 1) * CG].T
        ).astype(BF)
        per_g.append((wqkT, wvT, bqk, bv, wprojT))

    in_maps = []
    for b in range(B):
        idx = update_idx[b]
        mask = np.ones(NC, bool)
        mask[idx] = False
        keep = np.nonzero(mask)[0]
        xT = np.ascontiguousarray(x[b].T).astype(BF)
        for g in range(G):
            wqkT, wvT, bqk, bv, wprojT = per_g[g]
            hsel = slice(g * HPG, (g + 1) * HPG)
            kk = cache_k[b, hsel][:, keep, :]
            kkeepT = np.ascontiguousarray(
                kk.transpose(0, 2, 1).reshape(HPG * DH, NC - NX)).astype(BF)
            vk = cache_v[b, hsel][:, keep, :].transpose(1, 0, 2)
            vkeep = np.ascontiguousarray(
                np.concatenate(
                    [vk, np.ones((NC - NX, HPG, 1), np.float32)], axis=2
                ).reshape(NC - NX, HPG * VW)).astype(BF)
            in_maps.append(dict(
                xT=xT, wqkT=wqkT, wvT=wvT, bqk=bqk, bv=bv,
                kkeepT=kkeepT, vkeep=vkeep, wprojT=wprojT,
                ones8=np.ones((128, 8), BF),
            ))
    return in_maps


def kernel(x, update_idx, cache_k, cache_v, w_qkv, b_qkv, w_proj, b_proj):
    if "nc" not in _STATE:
        _STATE["nc"] = _build()
    nc = _STATE["nc"]
    _STATE["w_proj"] = np.asarray(w_proj, np.float32)
    b_proj = np.asarray(b_proj, np.float32)
    in_maps = _prep_in_maps(x, update_idx, cache_k, cache_v, w_qkv, b_qkv)
    res = bass_utils.run_bass_kernel_spmd(nc, in_maps, core_ids=list(range(N_CORES)))
    _STATE["last_results"] = res
    out = np.empty((B, NX, C), np.float32)
    for b in range(B):
        acc = (res.results[2 * b]["outT"].astype(np.float32)
               + res.results[2 * b + 1]["outT"].astype(np.float32))
        out[b] = acc.T + b_proj
    return out


# revision 3
# speedup vs baseline: 1.0092x; 1.0092x over previous
"""Decoder attention (QKV proj + KV-cache scatter + softmax attention + out
proj) on 8 trn2 cores — v3.

Core = (batch b, head-group g): b = core//2, g = core%2; 8 heads per core.
Softmax over a permutation of the key axis: attend over concat([k_new,
cache_keep]) — no on-device scatter (complement rows gathered host-side).

All matmuls bf16 (fp32 PSUM). Engine schedule built from HW microbenchmarks:
  - Scores (contraction DH=64) run as ROW-TILED CONCURRENT head pairs: heads
    2hp/2hp+1 live on partitions 0-63/64-127 of the q/k tiles, so their MMs
    land on PE row-groups 0-1/2-3 and overlap (measured 114 ns/MM vs 448
    serial, N=512).
  - All chained matmuls order chunk-columns INNERMOST so each stationary
    serves 2 adjacent MMs — amortizes LDWEIGHTS (measured 229 vs 302 ns/MM).
  - exp on ACT: 2 tiles of [128,1024] per (pair, jt) back to back.
  - attn@V with a ones-column appended to V (VW=65): softmax denominator
    accumulates free in the same PSUM tile; per-query reciprocal broadcast
    via gpsimd; DVE multiplies straight out of PSUM (no staging copy).

PSUM: shared [128,1024] f32 pool bufs=2 (4 banks) for scores/qkv/vproj/proj
+ avA/avB accumulators [65,1024] (2+2 banks) = 8 banks exactly.
"""

import sys
import os

for _p in ("/opt/trn_rl_repo", "/root/.axon_site/_ro/trn_rl_repo"):
    if os.path.isdir(_p) and _p not in sys.path:
        sys.path.insert(0, _p)
        break

import numpy as np
import ml_dtypes

import concourse.bacc as bacc
import concourse.mybir as mybir
import concourse.tile as tile
from concourse import bass_utils

B, NX, NC, C, H = 4, 1024, 2048, 1024, 16
DH = C // H                      # 64
G = 2                            # head groups
HPG = H // G                     # 8 heads per group
CG = HPG * DH                    # 512 channels per group
SCALE = DH ** -0.5
N_CORES = 8
NJ = NC // 128                   # 16 j-tiles
VW = DH + 1                      # 65
F32 = mybir.dt.float32
BF16 = mybir.dt.bfloat16
EXP = mybir.ActivationFunctionType.Exp

_STATE = {}


def _build(reps: int = 1):
    nc = bacc.Bacc("TRN2", target_bir_lowering=False, debug=False)

    xT_d = nc.dram_tensor("xT", [C, NX], BF16, kind="ExternalInput")
    wqkT_d = nc.dram_tensor("wqkT", [C, 2 * CG], BF16, kind="ExternalInput")
    wvT_d = nc.dram_tensor("wvT", [C, CG], BF16, kind="ExternalInput")
    bqk_d = nc.dram_tensor("bqk", [128, 8], F32, kind="ExternalInput")
    bv_d = nc.dram_tensor("bv", [128, CG], F32, kind="ExternalInput")
    kkeepT_d = nc.dram_tensor("kkeepT", [CG, NC - NX], BF16, kind="ExternalInput")
    vkeep_d = nc.dram_tensor("vkeep", [NC - NX, HPG * VW], BF16, kind="ExternalInput")
    wprojT_d = nc.dram_tensor("wprojT", [CG, C], BF16, kind="ExternalInput")
    ones_d = nc.dram_tensor("ones8", [128, 8], BF16, kind="ExternalInput")
    outT_d = nc.dram_tensor("outT", [C, NX], BF16, kind="ExternalOutput")

    with tile.TileContext(nc) as tc:
        with (
            tc.tile_pool(name="persist", bufs=1) as pp,
            tc.tile_pool(name="work", bufs=1) as wp,
            tc.tile_pool(name="wqkc", bufs=16) as wqkp,
            tc.tile_pool(name="etA", bufs=3) as epA,
            tc.tile_pool(name="etB", bufs=3) as epB,
            tc.tile_pool(name="nrm", bufs=2) as np_pool,
            tc.tile_pool(name="out_sb", bufs=1) as op,
            tc.tile_pool(name="ps", bufs=1, space="PSUM") as psp,
        ):
            q_t = [pp.tile([128, NX], BF16, tag=f"q{i}", name=f"q{i}") for i in range(4)]
            k_t = [pp.tile([128, NC], BF16, tag=f"k{i}", name=f"k{i}") for i in range(4)]
            v_t = [pp.tile([128, HPG * VW], BF16, tag=f"v{i}", name=f"v{i}")
                   for i in range(NJ)]
            a_t = [pp.tile([128, NX], BF16, tag=f"a{i}", name=f"a{i}") for i in range(4)]
            wpj_t = [pp.tile([128, NX], BF16, tag=f"wp{i}", name=f"wp{i}")
                     for i in range(4)]
            bqk_t = pp.tile([128, 8], F32, tag="bqk")
            bv_t = pp.tile([128, CG], F32, tag="bv")
            xT_t = [wp.tile([128, NX], BF16, tag=f"x{i}", name=f"x{i}") for i in range(8)]
            wv_t = [wp.tile([128, CG], BF16, tag=f"wv{i}", name=f"wv{i}")
                    for i in range(8)]

            nc.sync.dma_start(bqk_t[:], bqk_d.ap())
            nc.sync.dma_start(bv_t[:], bv_d.ap())

            def band_ps(i, name):
                """PSUM for the PE-only band (qkv/vproj/proj): alternate the
                two score tags so consecutive groups double-buffer."""
                return psp.tile([128, NX], F32, tag=("sA" if i % 2 == 0 else "sB"),
                                bufs=1, name=name)

            def qk_mtile(m, bi, dma_only=False, chunks=None):
                """m 0-3: q m-tile; 4-7: k m-tile. Chunk-cols innermost so
                each wqk chunk stationary serves 2 adjacent MMs."""
                if chunks is None:
                    chunks = []
                    for kk in range(8):
                        wc = wqkp.tile([128, 128], BF16, tag="wqkc",
                                       name=f"wqkc{m}_{kk}")
                        nc.sync.dma_start(
                            wc[:],
                            wqkT_d[kk * 128:(kk + 1) * 128, m * 128:(m + 1) * 128])
                        chunks.append(wc)
                    if dma_only:
                        return chunks
                qps = band_ps(bi, f"qps{m}")
                for kk in range(8):
                    for cch in range(2):
                        nc.tensor.matmul(
                            qps[:, cch * 512:(cch + 1) * 512],
                            chunks[kk][:],
                            xT_t[kk][:, cch * 512:(cch + 1) * 512],
                            start=(kk == 0), stop=(kk == 7),
                        )
                dest = q_t[m][:] if m < 4 else k_t[m - 4][:, 0:NX]
                nc.vector.tensor_scalar_add(dest, qps[:], bqk_t[:, m:m + 1])
                return None

            def body():
                # ---- DMAs ordered by first use; first q m-tile jumps queue ----
                nc.sync.dma_start(xT_t[0][:], xT_d[0:128, :])
                m0_chunks = qk_mtile(0, 0, dma_only=True)
                for i in range(1, 8):
                    nc.sync.dma_start(xT_t[i][:], xT_d[i * 128:(i + 1) * 128, :])
                nc.sync.dma_start(k_t[0][:, NX:NC], kkeepT_d[0:128, :])
                m4_chunks = qk_mtile(4, 0, dma_only=True)
                for i in range(8):
                    nc.sync.dma_start(wv_t[i][:], wvT_d[i * 128:(i + 1) * 128, :])
                for m in range(NJ // 2):
                    nc.sync.dma_start(
                        v_t[m][:].rearrange("p (h w) -> p h w", w=VW)[:, :, DH],
                        ones_d.ap(),
                    )

                bi = 0
                qk_mtile(0, bi, chunks=m0_chunks)
                bi += 1
                qk_mtile(4, bi, chunks=m4_chunks)
                bi += 1

                # bulk loads, behind the critical path
                for i in range(1, 4):
                    nc.sync.dma_start(k_t[i][:, NX:NC],
                                      kkeepT_d[i * 128:(i + 1) * 128, :])
                for j in range(NJ // 2, NJ):
                    r0 = (j - NJ // 2) * 128
                    nc.sync.dma_start(v_t[j][:], vkeep_d[r0:r0 + 128, :])
                for kk in range(4):
                    nc.sync.dma_start(wpj_t[kk][:],
                                      wprojT_d[kk * 128:(kk + 1) * 128, :])

                # ---- rest of the PE band: remaining q/k m-tiles + vproj ----
                def v_mtile(m, bi):
                    vps = band_ps(bi, f"vps{m}")
                    for kk in range(8):
                        nc.tensor.matmul(
                            vps[:, 0:CG],
                            xT_t[kk][:, m * 128:(m + 1) * 128],
                            wv_t[kk][:],
                            start=(kk == 0), stop=(kk == 7),
                        )
                    for h in range(HPG):
                        nc.vector.tensor_add(
                            v_t[m][:, h * VW:h * VW + DH],
                            vps[:, h * DH:(h + 1) * DH],
                            bv_t[:, h * DH:(h + 1) * DH],
                        )

                for m in (1, 5, 2, 6, 3, 7):
                    qk_mtile(m, bi)
                    bi += 1
                for m in range(8):
                    v_mtile(m, bi)
                    bi += 1

                # ---- attention on head pairs (rowtile concurrency); no other
                # PE work interleaved: per-jt chain is scores -> exp -> AV with
                # single-buffer score tiles and alternating head order so the
                # buffer wait is always on the exp that finished first ----
                def attn_pair(hp):
                    avA = psp.tile([VW, NX], F32, tag="avA", bufs=1, name=f"avA{hp}")
                    avB = psp.tile([VW, NX], F32, tag="avB", bufs=1, name=f"avB{hp}")
                    ets = {}

                    def emit_av(j, first, last):
                        for idx, av in ((0, avA), (1, avB)):
                            et = ets[(idx, j)]
                            for cch in range(2):
                                nc.tensor.matmul(
                                    av[:, cch * 512:(cch + 1) * 512],
                                    v_t[j][:, (2 * hp + idx) * VW:(2 * hp + idx + 1) * VW],
                                    et[:, cch * 512:(cch + 1) * 512],
                                    start=first, stop=last,
                                )

                    for step, j in enumerate(range(NJ)):
                        sA = psp.tile([128, NX], F32, tag="sA", bufs=1,
                                      name=f"sA{hp}_{j}")
                        sB = psp.tile([128, NX], F32, tag="sB", bufs=1,
                                      name=f"sB{hp}_{j}")
                        # interleave the pair's chunks (concurrent row groups);
                        # alternate which head leads to match exp completion
                        first = step % 2
                        heads = [(0, sA, 0), (1, sB, 64)]
                        order = [heads[first], heads[1 - first]]
                        for cch in range(2):
                            for idx, sX, po in order:
                                nc.tensor.matmul(
                                    sX[:, cch * 512:(cch + 1) * 512],
                                    k_t[hp][po:po + 64, j * 128:(j + 1) * 128],
                                    q_t[hp][po:po + 64, cch * 512:(cch + 1) * 512],
                                    start=True, stop=True,
                                )
                        etA = epA.tile([128, NX], BF16, tag="etA", name=f"etA{hp}_{j}")
                        etB = epB.tile([128, NX], BF16, tag="etB", name=f"etB{hp}_{j}")
                        ets[(0, j)] = etA
                        ets[(1, j)] = etB
                        for idx, sX, po in order:
                            nc.scalar.activation(ets[(idx, j)][:], sX[:], EXP,
                                                 scale=SCALE)
                        if step >= 1:
                            emit_av(j - 1, step == 1, False)
                    emit_av(NJ - 1, False, True)

                    for idx, av in ((0, avA), (1, avB)):
                        recip = np_pool.tile([1, NX], F32, tag="recip", bufs=2)
                        nc.vector.reciprocal(recip[:], av[DH:VW, :])
                        rb = np_pool.tile([64, NX], F32, tag="rb", bufs=2)
                        nc.gpsimd.partition_broadcast(rb[:], recip[:])
                        nc.vector.tensor_mul(
                            a_t[hp][idx * 64:(idx + 1) * 64, :],
                            av[0:DH, :], rb[:])

                for hp in range(4):
                    attn_pair(hp)

                # ---- output projection (partial; host sums the two groups) ----
                for m in range(8):
                    pps = band_ps(bi, f"pps{m}")
                    bi += 1
                    for kk in range(4):
                        for cch in range(2):
                            nc.tensor.matmul(
                                pps[:, cch * 512:(cch + 1) * 512],
                                wpj_t[kk][:, m * 128:(m + 1) * 128],
                                a_t[kk][:, cch * 512:(cch + 1) * 512],
                                start=(kk == 0), stop=(kk == 3),
                            )
                    ot = op.tile([128, NX], BF16, tag="ot", bufs=2)
                    nc.vector.tensor_copy(ot[:], pps[:])
                    nc.sync.dma_start(outT_d[m * 128:(m + 1) * 128, :], ot[:])

            if reps == 1:
                body()
            else:
                hints = (
                    mybir.EngineType.PE,
                    mybir.EngineType.Activation,
                    mybir.EngineType.DVE,
                    mybir.EngineType.SP,
                    mybir.EngineType.Pool,
                )
                with tc.For_i(0, reps, 1, hint_engines=hints):
                    body()

    nc.compile()
    return nc


def _prep_in_maps(x, update_idx, cache_k, cache_v, w_qkv, b_qkv):
    """Host-side sharding: 8 per-core input dicts (bf16 activations/weights)."""
    BF = ml_dtypes.bfloat16
    x = np.asarray(x, np.float32)
    update_idx = np.asarray(update_idx)
    cache_k = np.asarray(cache_k, np.float32)
    cache_v = np.asarray(cache_v, np.float32)
    w_qkv = np.asarray(w_qkv, np.float32)
    b_qkv = np.asarray(b_qkv, np.float32)

    per_g = []
    for g in range(G):
        qs = slice(g * CG, (g + 1) * CG)
        ks = slice(C + g * CG, C + (g + 1) * CG)
        vs = slice(2 * C + g * CG, 2 * C + (g + 1) * CG)
        wqkT = np.ascontiguousarray(
            np.concatenate([w_qkv[qs], w_qkv[ks]], 0).T).astype(BF)
        wvT = np.ascontiguousarray(w_qkv[vs].T).astype(BF)
        bqk = np.ascontiguousarray(
            np.concatenate([b_qkv[qs], b_qkv[ks]]).reshape(8, 128).T)
        bv = np.broadcast_to(b_qkv[vs][None, :], (128, CG)).copy()
        wprojT = np.ascontiguousarray(
            np.asarray(_STATE["w_proj"], np.float32)[:, g * CG:(g + 1) * CG].T
        ).astype(BF)
        per_g.append((wqkT, wvT, bqk, bv, wprojT))

    in_maps = []
    for b in range(B):
        idx = update_idx[b]
        mask = np.ones(NC, bool)
        mask[idx] = False
        keep = np.nonzero(mask)[0]
        xT = np.ascontiguousarray(x[b].T).astype(BF)
        for g in range(G):
            wqkT, wvT, bqk, bv, wprojT = per_g[g]
            hsel = slice(g * HPG, (g + 1) * HPG)
            kk = cache_k[b, hsel][:, keep, :]
            kkeepT = np.ascontiguousarray(
                kk.transpose(0, 2, 1).reshape(HPG * DH, NC - NX)).astype(BF)
            vk = cache_v[b, hsel][:, keep, :].transpose(1, 0, 2)
            vkeep = np.ascontiguousarray(
                np.concatenate(
                    [vk, np.ones((NC - NX, HPG, 1), np.float32)], axis=2
                ).reshape(NC - NX, HPG * VW)).astype(BF)
            in_maps.append(dict(
                xT=xT, wqkT=wqkT, wvT=wvT, bqk=bqk, bv=bv,
                kkeepT=kkeepT, vkeep=vkeep, wprojT=wprojT,
                ones8=np.ones((128, 8), BF),
            ))
    return in_maps


def kernel(x, update_idx, cache_k, cache_v, w_qkv, b_qkv, w_proj, b_proj):
    if "nc" not in _STATE:
        _STATE["nc"] = _build()
    nc = _STATE["nc"]
    _STATE["w_proj"] = np.asarray(w_proj, np.float32)
    b_proj = np.asarray(b_proj, np.float32)
    in_maps = _prep_in_maps(x, update_idx, cache_k, cache_v, w_qkv, b_qkv)
    res = bass_utils.run_bass_kernel_spmd(nc, in_maps, core_ids=list(range(N_CORES)))
    _STATE["last_results"] = res
    out = np.empty((B, NX, C), np.float32)
    for b in range(B):
        acc = (res.results[2 * b]["outT"].astype(np.float32)
               + res.results[2 * b + 1]["outT"].astype(np.float32))
        out[b] = acc.T + b_proj
    return out


# revision 5
# speedup vs baseline: 1.4225x; 1.4095x over previous
"""Decoder attention (QKV proj + KV-cache scatter + full softmax attention + out proj)
on 8 Trainium2 cores.

Sharding: core = (batch b, head-group g).  b = core//2, g = core%2; each core
handles 8 of the 16 heads for one batch element.

Key algorithmic point: softmax + attn@V are invariant to a permutation of the
key axis, so the reference's masked_scatter of new K/V into the cache is
equivalent to attending over concat([k_new, cache_keep]) where cache_keep are
the cache rows NOT in update_idx (complement set, gathered host-side during
sharding).  No on-device scatter is needed.

Device kernel (per core), all layouts chosen so no on-device transpose is
ever needed:
  - QKV:   qkT  (c', n)  = w_qkT.T @ xT      (c' = 8 q-heads*64 then 8 k-heads*64)
           v    (n,  c') = xT.T @ w_vT
  - attn:  scoresT (j, n) = k_eff.T-chunks @ qT ; exp on ACT (scale folded in);
           attn@V with V augmented by a ones-column -> softmax denominator
           accumulates for free in the same PSUM tile (row 64).
  - norm:  reciprocal + gpsimd partition_broadcast + DVE multiply.
  - proj:  outT (c_out, n) = w_projT.T @ attn_catT   (partial; host sums the
           two head-group partials per batch and adds b_proj).

All matmuls run in bf16 (fp32 PSUM accumulation): measured ~55us faster
than float32r on HW -- f32r streams slower than the cost model's 1
cycle/row; bf16 does not -- and input DMA bytes halve. Rel err 4.0e-3.
"""

import sys

import os

for _p in ("/opt/trn_rl_repo", "/root/.axon_site/_ro/trn_rl_repo"):
    if os.path.isdir(_p) and _p not in sys.path:
        sys.path.insert(0, _p)
        break

import numpy as np

import concourse.bacc as bacc
import concourse.mybir as mybir
import concourse.tile as tile
from concourse import bass_utils

B, NX, NC, C, H = 4, 1024, 2048, 1024, 16
DH = C // H                      # 64
G = 2                            # head groups (tensor-parallel factor)
HPG = H // G                     # 8 heads per group
CG = HPG * DH                    # 512 channels per group
SCALE = DH ** -0.5
N_CORES = 8
F32 = mybir.dt.float32
F32R = mybir.dt.float32r
BF16 = mybir.dt.bfloat16
EXP = mybir.ActivationFunctionType.Exp

# matmul dtypes per stage (float32r = full-rate, ~tf32 accuracy; accumulation
# is always fp32 in PSUM)
DT_QKV = F32R
DT_SCORES = F32R
DT_AV = F32R
DT_PROJ = F32R

_STATE = {}


def _r(ap, dt):
    return ap.bitcast(dt) if dt is not F32 else ap


def _build(reps: int = 1, exp_mode: str = "act"):
    """Build + compile the per-core Bass program.

    reps > 1 wraps the whole computation in an on-device hardware loop --
    used only for timing (amortizes host->device dispatch latency).
    """
    nc = bacc.Bacc("TRN2", target_bir_lowering=False, debug=False)

    xT_d = nc.dram_tensor("xT", [C, NX], BF16, kind="ExternalInput")
    wqkT_d = nc.dram_tensor("wqkT", [C, 2 * CG], BF16, kind="ExternalInput")
    wvT_d = nc.dram_tensor("wvT", [C, CG], BF16, kind="ExternalInput")
    bqk_d = nc.dram_tensor("bqk", [128, 8], F32, kind="ExternalInput")
    bv_d = nc.dram_tensor("bv", [128, CG], F32, kind="ExternalInput")
    kkeepT_d = nc.dram_tensor("kkeepT", [CG, NC - NX], BF16, kind="ExternalInput")
    vkeep_d = nc.dram_tensor("vkeep", [NC - NX, HPG * (DH + 1)], BF16, kind="ExternalInput")
    wprojT_d = nc.dram_tensor("wprojT", [CG, C], BF16, kind="ExternalInput")
    ones_d = nc.dram_tensor("ones8", [128, 8], BF16, kind="ExternalInput")
    outT_d = nc.dram_tensor("outT", [C, NX], BF16, kind="ExternalOutput")

    NJ = NC // 128               # 16 j-tiles over the effective kv length
    VW = DH + 1                  # 65: v columns + ones column per head

    with tile.TileContext(nc) as tc:
        with (
            tc.tile_pool(name="persist", bufs=1) as pp,
            tc.tile_pool(name="work", bufs=1) as wp,
            tc.tile_pool(name="wqkc", bufs=24) as wqkp,
            tc.tile_pool(name="attn", bufs=4) as ep,
            tc.tile_pool(name="nrm", bufs=2) as np_pool,
            tc.tile_pool(name="out_sb", bufs=1) as op,
            tc.tile_pool(name="ps", bufs=1, space="PSUM") as psp,
        ):
            # ---- persistent tiles ----
            q_t = [pp.tile([128, NX], BF16, tag=f"q{i}", name=f"q{i}") for i in range(4)]
            k_t = [pp.tile([128, NC], BF16, tag=f"k{i}", name=f"k{i}") for i in range(4)]
            v_t = [pp.tile([128, HPG * VW], BF16, tag=f"v{i}", name=f"v{i}") for i in range(NJ)]
            a_t = [pp.tile([128, NX], BF16, tag=f"a{i}", name=f"a{i}") for i in range(4)]
            wpj_t = [pp.tile([128, NX], BF16, tag=f"wpj{i}", name=f"wpj{i}")
                     for i in range(4)]
            bqk_t = pp.tile([128, 8], F32, tag="bqk")
            bv_t = pp.tile([128, CG], F32, tag="bv")
            xT_t = [wp.tile([128, NX], BF16, tag=f"x{i}", name=f"x{i}") for i in range(8)]
            wv_t = [wp.tile([128, CG], BF16, tag=f"wv{i}", name=f"wv{i}") for i in range(8)]

            nc.sync.dma_start(bqk_t[:], bqk_d.ap())
            nc.sync.dma_start(bv_t[:], bv_d.ap())

            def body():
                # ---- priority DMAs, ordered by first use; the first head
                # pair's w_qk chunks jump the queue so the QKV matmuls start
                # ~2us in instead of waiting behind the bulk loads ----
                pre_chunks = {}

                def prefetch_chunks(m):
                    for kk in range(8):
                        wqk_c = wqkp.tile([128, 128], BF16, tag="wqkc",
                                          name=f"wqkcP{m}_{kk}")
                        nc.sync.dma_start(
                            wqk_c[:],
                            wqkT_d[kk * 128:(kk + 1) * 128, m * 128:(m + 1) * 128],
                        )
                        pre_chunks[(m, kk)] = wqk_c

                nc.sync.dma_start(xT_t[0][:], xT_d[0:128, :])
                prefetch_chunks(0)
                nc.sync.dma_start(k_t[0][:, NX:NC], kkeepT_d[0:128, :])
                prefetch_chunks(4)
                for i in range(1, 8):
                    nc.sync.dma_start(xT_t[i][:], xT_d[i * 128:(i + 1) * 128, :])
                # ones columns of the v tiles (bias adds never touch them;
                # vkeep rows arrive with ones baked in)
                for m in range(NJ // 2):
                    nc.sync.dma_start(
                        v_t[m][:].rearrange("p (h w) -> p h w", w=VW)[:, :, DH],
                        ones_d.ap(),
                    )
                for i in range(8):
                    nc.sync.dma_start(wv_t[i][:], wvT_d[i * 128:(i + 1) * 128, :])
                for i in range(1, 4):
                    nc.sync.dma_start(k_t[i][:, NX:NC], kkeepT_d[i * 128:(i + 1) * 128, :])
                for j in range(NJ // 2, NJ):
                    r0 = (j - NJ // 2) * 128
                    nc.sync.dma_start(v_t[j][:], vkeep_d[r0:r0 + 128, :])
                for kkk in range(4):
                    nc.sync.dma_start(wpj_t[kkk][:],
                                      wprojT_d[kkk * 128:(kkk + 1) * 128, :])

                def qk_thunks(i):
                    """Matmul/bias thunks for q m-tile i and k m-tile 4+i,
                    drained one per attention j-step.  Each thunk DMAs its own
                    64KB w_qk chunk (contiguous) then consumes it."""
                    for m in (i, 4 + i):
                        qps = psp.tile([128, NX], F32, tag="qps", bufs=1, name=f"qps{m}")
                        for kk in range(8):
                            def mm(m=m, kk=kk, qps=qps):
                                wqk_c = pre_chunks.pop((m, kk), None)
                                if wqk_c is None:
                                    wqk_c = wqkp.tile([128, 128], BF16, tag="wqkc",
                                                      name=f"wqkc{m}_{kk}")
                                    nc.sync.dma_start(
                                        wqk_c[:],
                                        wqkT_d[kk * 128:(kk + 1) * 128, m * 128:(m + 1) * 128],
                                    )
                                for cch in range(2):
                                    nc.tensor.matmul(
                                        qps[:, cch * 512:(cch + 1) * 512],
                                        wqk_c[:],
                                        xT_t[kk][:, cch * 512:(cch + 1) * 512],
                                        start=(kk == 0),
                                        stop=(kk == 7),
                                    )
                            yield mm
                        def bias(m=m, qps=qps):
                            if m < 4:
                                dest = q_t[m][:]
                            else:
                                dest = k_t[m - 4][:, 0:NX]
                            nc.vector.tensor_scalar_add(dest, qps[:], bqk_t[:, m:m + 1])
                        yield bias

                pending = []

                def drain(n):
                    for _ in range(n):
                        if not pending:
                            return
                        pending.pop(0)()

                # qk pair 0 runs up front (attention depends on it)
                for th in qk_thunks(0):
                    th()

                def v_thunks():
                    """v projection m-tiles as drain thunks (9 per m: 8 matmuls
                    + the bias/scatter finisher), interleaved into head 0."""
                    for m in range(8):
                        vps = psp.tile([128, NX], F32, tag="qps", bufs=1, name=f"vps{m}")
                        for kk in range(8):
                            def mm(m=m, kk=kk, vps=vps):
                                nc.tensor.matmul(
                                    vps[:, 0:CG],
                                    xT_t[kk][:, m * 128:(m + 1) * 128],
                                    wv_t[kk][:],
                                    start=(kk == 0),
                                    stop=(kk == 7),
                                )
                            yield mm
                        def fin(m=m, vps=vps):
                            for h in range(HPG):
                                nc.vector.tensor_add(
                                    v_t[m][:, h * VW:h * VW + DH],
                                    vps[:, h * DH:(h + 1) * DH],
                                    bv_t[:, h * DH:(h + 1) * DH],
                                )
                        yield fin

                # ---- phase 2: attention; j loop software-pipelined (av for
                # j-1 after scores/exp for j) with leftover QKV matmuls
                # drained one per j-step to fill PE idle time ----
                def attn_head(h, av_lag=1, drain_per_j=1, j_order=None):
                    hp, po = h // 2, 64 * (h % 2)
                    jo = list(j_order) if j_order is not None else list(range(NJ))
                    av = psp.tile([VW, NX], F32, tag="av", bufs=1, name=f"av{h}")
                    ets = [None] * NJ

                    def emit_av(j):
                        for cch in range(2):
                            nc.tensor.matmul(
                                av[:, cch * 512:(cch + 1) * 512],
                                v_t[j][:, h * VW:(h + 1) * VW],
                                ets[j][:, cch * 512:(cch + 1) * 512],
                                start=(j == jo[0]),
                                stop=(j == jo[-1]),
                            )

                    for step, j in enumerate(jo):
                        sps = psp.tile([128, NX], F32, tag="sps", bufs=2, name=f"sps{h}_{j}")
                        for cch in range(2):
                            nc.tensor.matmul(
                                sps[:, cch * 512:(cch + 1) * 512],
                                k_t[hp][po:po + 64, j * 128:(j + 1) * 128],
                                q_t[hp][po:po + 64, cch * 512:(cch + 1) * 512],
                                start=True,
                                stop=True,
                            )
                        et = ep.tile([128, NX], BF16, tag="et", name=f"et{h}_{j}")
                        ets[j] = et
                        if exp_mode == "act":
                            nc.scalar.activation(et[:], sps[:], EXP, scale=SCALE)
                        else:
                            # timing probe only: wrong math, same data movement
                            nc.vector.tensor_copy(et[:], sps[:])
                        drain(drain_per_j)
                        if step >= av_lag:
                            emit_av(jo[step - av_lag])
                    for step in range(NJ - av_lag, NJ):
                        emit_av(jo[step])
                    avs = np_pool.tile([VW, NX], F32, tag="avs", bufs=2, name=f"avs{h}")
                    nc.vector.tensor_copy(avs[:], av[:])
                    recip = np_pool.tile([1, NX], F32, tag="recip", bufs=1)
                    nc.vector.reciprocal(recip[:], avs[DH:VW, :])
                    rb = np_pool.tile([64, NX], F32, tag="rb", bufs=1)
                    nc.gpsimd.partition_broadcast(rb[:], recip[:])
                    nc.vector.tensor_mul(a_t[hp][po:po + 64, :], avs[0:DH, :], rb[:])

                pending.extend(v_thunks())
                # head 0 visits the cache-half key tiles first: their V rows
                # come from DMA, so attention starts before the V projection
                # (draining concurrently) has produced anything
                attn_head(0, av_lag=2, drain_per_j=9,
                          j_order=list(range(NJ // 2, NJ)) + list(range(NJ // 2)))
                for h in range(1, HPG):
                    if h in (2, 4, 6):
                        drain(80)            # pair (h//2) must be complete
                    if h in (1, 3, 5):
                        pending.extend(qk_thunks((h + 1) // 2))
                    attn_head(h)
                drain(80)

                # ---- phase 3: output projection (partial: this head group);
                # wproj prefetched to SBUF; kk-outer so each stationary
                # chunk serves both column-chunk MMs adjacently ----
                for m in range(8):
                    pps = psp.tile([128, NX], F32, tag="sps", bufs=2, name=f"pps{m}")
                    for kk in range(4):
                        for cch in range(2):
                            nc.tensor.matmul(
                                pps[:, cch * 512:(cch + 1) * 512],
                                wpj_t[kk][:, m * 128:(m + 1) * 128],
                                a_t[kk][:, cch * 512:(cch + 1) * 512],
                                start=(kk == 0),
                                stop=(kk == 3),
                            )
                    ot = op.tile([128, NX], BF16, tag="ot", bufs=2)
                    if m % 2 == 0:
                        nc.vector.tensor_copy(ot[:], pps[:])
                    else:
                        # ACT is idle during proj; split PSUM evacuation
                        nc.scalar.activation(
                            ot[:], pps[:], mybir.ActivationFunctionType.Identity
                        )
                    nc.sync.dma_start(outT_d[m * 128:(m + 1) * 128, :], ot[:])

            if reps == 1:
                body()
            else:
                hints = (
                    mybir.EngineType.PE,
                    mybir.EngineType.Activation,
                    mybir.EngineType.DVE,
                    mybir.EngineType.SP,
                )
                with tc.For_i(0, reps, 1, hint_engines=hints):
                    body()

    nc.compile()
    return nc


def _get_nc():
    if "nc" not in _STATE:
        _STATE["nc"] = _build()
    return _STATE["nc"]


def _prep_in_maps(x, update_idx, cache_k, cache_v, w_qkv, b_qkv):
    """Host-side sharding: build the 8 per-core input dicts."""
    x = np.asarray(x, np.float32)
    update_idx = np.asarray(update_idx)
    cache_k = np.asarray(cache_k, np.float32)
    cache_v = np.asarray(cache_v, np.float32)
    w_qkv = np.asarray(w_qkv, np.float32)
    b_qkv = np.asarray(b_qkv, np.float32)

    per_g = []
    for g in range(G):
        qs = slice(g * CG, (g + 1) * CG)
        ks = slice(C + g * CG, C + (g + 1) * CG)
        vs = slice(2 * C + g * CG, 2 * C + (g + 1) * CG)
        import ml_dtypes
        wqkT = np.ascontiguousarray(
            np.concatenate([w_qkv[qs], w_qkv[ks]], 0).T
        ).astype(ml_dtypes.bfloat16)                                                    # (C, 2CG)
        wvT = np.ascontiguousarray(w_qkv[vs].T).astype(ml_dtypes.bfloat16)              # (C, CG)
        bqk = np.ascontiguousarray(
            np.concatenate([b_qkv[qs], b_qkv[ks]]).reshape(8, 128).T
        )                                                    # (128, 8)
        bv = np.broadcast_to(b_qkv[vs][None, :], (128, CG)).copy()
        per_g.append((wqkT, wvT, bqk, bv))

    in_maps = []
    for b in range(B):
        idx = update_idx[b]
        mask = np.ones(NC, bool)
        mask[idx] = False
        keep = np.nonzero(mask)[0]                           # (NC-NX,) sorted
        xT = np.ascontiguousarray(x[b].T).astype(__import__("ml_dtypes").bfloat16)                    # (C, NX)
        for g in range(G):
            wqkT, wvT, bqk, bv = per_g[g]
            hsel = slice(g * HPG, (g + 1) * HPG)
            kk = cache_k[b, hsel][:, keep, :]                # (HPG, NC-NX, DH)
            import ml_dtypes
            kkeepT = np.ascontiguousarray(
                kk.transpose(0, 2, 1).reshape(HPG * DH, NC - NX)
            ).astype(ml_dtypes.bfloat16)
            vk = cache_v[b, hsel][:, keep, :].transpose(1, 0, 2)  # (NC-NX, HPG, DH)
            vkeep = np.ascontiguousarray(
                np.concatenate(
                    [vk, np.ones((NC - NX, HPG, 1), np.float32)], axis=2
                ).reshape(NC - NX, HPG * (DH + 1))
            ).astype(ml_dtypes.bfloat16)
            wprojT = np.asarray(_STATE["wprojT"][g], __import__("ml_dtypes").bfloat16)
            in_maps.append(
                dict(
                    xT=xT, wqkT=wqkT, wvT=wvT, bqk=bqk, bv=bv,
                    kkeepT=kkeepT, vkeep=vkeep, wprojT=wprojT,
                    ones8=np.ones((128, 8), __import__('ml_dtypes').bfloat16),
                )
            )
    return in_maps


def kernel(x, update_idx, cache_k, cache_v, w_qkv, b_qkv, w_proj, b_proj):
    nc = _get_nc()
    w_proj = np.asarray(w_proj, np.float32)
    b_proj = np.asarray(b_proj, np.float32)
    _STATE["wprojT"] = [
        np.ascontiguousarray(w_proj[:, g * CG:(g + 1) * CG].T) for g in range(G)
    ]
    in_maps = _prep_in_maps(x, update_idx, cache_k, cache_v, w_qkv, b_qkv)
    res = bass_utils.run_bass_kernel_spmd(nc, in_maps, core_ids=list(range(N_CORES)))
    _STATE["last_results"] = res
    out = np.empty((B, NX, C), np.float32)
    for b in range(B):
        acc = (res.results[2 * b]["outT"].astype(np.float32)
               + res.results[2 * b + 1]["outT"].astype(np.float32))
        out[b] = acc.T + b_proj
    return out

